# revision 72
# baseline (speedup 1.0000x reference)
"""Multi-head attention (B=2, S=2048, D=1024, H=16) on 8 Trainium2 cores.

Sharding: pure tensor-parallel over heads (Megatron): core c owns heads
{2c, 2c+1} (d_local = 128 columns of Wq/Wk/Wv, 128 rows of Wo) and
processes BOTH batches. Each core emits a [2, 2048, 1024] partial output
(row-parallel Wo); the host sums the 8 partials per batch.

v2 schedule (86.7us vs the 89.9us baseline, TimelineSim cost model):
- A dummy exp at t~0 pulls the 1.3us activation-table load off the
  first real exp's critical path; the tiny mask DMAs go out on the
  GPSIMD SWDGE queue ahead of the Pool memsets so they beat the big
  stream transfers into the serial DMA queue.
- DMA order puts the attention-gating tensors first (Wk+Wq packed
  partition-major in one DMA, K-A, Q-A chunk 0); V streams in column
  chunks and all projections run as single-PSUM-bank steps through a
  shared 2-bank aux rotation, scheduled into the attention cadence by
  iteration index (smap).
- attend(0) processes batch A with both heads of chunk 0 interleaved
  (early scores cover the V-stream wait) and batch B's chunk 0
  appended as tail groups; attend(1) covers B chunks 1-3. Out-proj
  tiles fire as each chunk's softmax normalizes, spread so the final
  phase stays below the ACT+DVE evacuation capacity; only chunk B-c3's
  4 tiles drain through a deep PSUM rotation after the loop.
- Inner attention loop emits [scores(t) | exp | extras | ctx(t)] so
  foreign PE work hides the exp latency.

Precision: fp16 streams/weights, fp32 PSUM accumulation (rel err ~8e-4).
PSUM: scores rotation 4 + ctx accumulators 2 + shared aux 2 = 8 banks.
"""
import sys
if "/opt/trn_rl_repo" not in sys.path:
    sys.path.insert(0, "/opt/trn_rl_repo")
import os
import time
import numpy as np

B, SQ, SK, D, H, HD = 2, 2048, 2048, 1024, 16, 64
NEG = -1.0e6
N_CORES = 8
DL = 128          # d_local: 2 heads * 64
KD = D // 128     # contraction tiles over D

_NC_CACHE = {}
last_results = None
last_exec_wall_s = None


def _build(KTA, KTB, LCA=None, LCB=None):
    import concourse.bass as bass  # noqa: F401
    import concourse.tile as tile
    from concourse import bacc, mybir

    f32 = mybir.dt.float32
    f16 = mybir.dt.float16
    EXP = mybir.ActivationFunctionType.Exp

    LKA, LKB = KTA * 128, KTB * 128
    LCA = LKA if LCA is None else LCA
    LCB = LKB if LCB is None else LCB

    nc = bacc.Bacc("TRN2", target_bir_lowering=False, debug=False,
                   num_devices=N_CORES)
    xqT = nc.dram_tensor("xqT", [2, D, SQ], f16, kind="ExternalInput")
    xkTA = nc.dram_tensor("xkTA", [D, LKA], f16, kind="ExternalInput")
    xvTA = nc.dram_tensor("xvTA", [D, LKA], f16, kind="ExternalInput")
    xkTB = nc.dram_tensor("xkTB", [D, LKB], f16, kind="ExternalInput")
    xvTB = nc.dram_tensor("xvTB", [D, LKB], f16, kind="ExternalInput")
    # host-packed partition-major weights: one big contiguous DMA each
    wkq = nc.dram_tensor("wkq", [128, KD * 256], f16, kind="ExternalInput")
    wv = nc.dram_tensor("wv", [128, KD * 128], f16, kind="ExternalInput")
    wo = nc.dram_tensor("wo", [DL, D], f16, kind="ExternalInput")
    maskA = nc.dram_tensor("maskA", [128, KTA], f32, kind="ExternalInput")
    maskB = nc.dram_tensor("maskB", [128, KTB], f32, kind="ExternalInput")
    out = nc.dram_tensor("out", [2, SQ, D], f16, kind="ExternalOutput")

    with tile.TileContext(nc) as tc:
        with tc.tile_pool(name="singles", bufs=1) as sg:
            wkq_sb = sg.tile([128, KD, 256], f16)
            wv_sb = sg.tile([128, KD, 128], f16)
            wo_sb = sg.tile([128, D], f16)
            maskA_sb = sg.tile([128, KTA], f32)
            maskB_sb = sg.tile([128, KTB], f32)
            kt_sb = {0: sg.tile([128, LKA], f16, name="ktA"),
                     1: sg.tile([128, LKB], f16, name="ktB")}
            qt_sb = {0: sg.tile([128, SQ], f16, name="qtA"),
                     1: sg.tile([128, SQ], f16, name="qtB")}
            v_sb = {0: sg.tile([128, KTA, 2, 128], f16, name="vA"),
                    1: sg.tile([128, KTB, 2, 128], f16, name="vB")}
            ctx_sb = {0: sg.tile([128, SQ], f16, name="ctxA"),
                      1: sg.tile([128, SQ], f16, name="ctxB")}
            warm_sb = sg.tile([128, 256], f16)

            KT = {0: KTA, 1: KTB}
            mask_sb = {0: maskA_sb, 1: maskB_sb}

            strm_cm = tc.tile_pool(name="streams", bufs=1)
            strm = strm_cm.__enter__()
            xk = {b: strm.tile([128, KD, 128 * KT[b]], f16, name=f"xk{b}")
                  for b in (0, 1)}
            xq = {b: strm.tile([128, KD, SQ], f16, name=f"xq{b}")
                  for b in (0, 1)}
            xv = {b: strm.tile([128, KD, 128 * KT[b]], f16, name=f"xv{b}")
                  for b in (0, 1)}
            LC = {0: LCA, 1: LCB}

            # ---- GPSIMD preamble: warm tile first (gates PE fills and the
            # ACT-table prewarm), then the tiny mask DMAs (so their SWDGE
            # descriptors beat the big stream transfers into the DMA queue),
            # then V'' ones-columns and stream tail zero-fill ----
            nc.vector.memset(warm_sb, 0.0)
            nc.gpsimd.dma_start(out=maskA_sb, in_=maskA[:, :])
            nc.gpsimd.dma_start(out=maskB_sb, in_=maskB[:, :])
            nc.gpsimd.memset(v_sb[0], 1.0)
            nc.gpsimd.memset(v_sb[1], 1.0)
            for b in (0, 1):
                if LC[b] < 128 * KT[b]:
                    nc.gpsimd.memset(xk[b][:, :, LC[b]:], 0.0)
                    nc.gpsimd.memset(xv[b][:, :, LC[b]:], 0.0)
            # dummy exp at t~0: pulls the 1.3us activation-table load off
            # the first real exp's critical path
            nc.scalar.activation(warm_sb[:, 255:256], warm_sb[:, 0:1], EXP,
                                 scale=1.0)

            # ---- SP/HWDGE input DMAs, arrival order = need order ----
            def dma_q_col(b, ci):
                nc.sync.dma_start(
                    out=xq[b][:, :, ci * 512:(ci + 1) * 512],
                    in_=xqT[b, :, ci * 512:(ci + 1) * 512]
                    .rearrange("(k p) j -> p k j", p=128))

            nc.sync.dma_start(
                out=wkq_sb.rearrange("p k j -> p (k j)"), in_=wkq[:, :])
            KC0 = min(512, LCA)
            nc.sync.dma_start(
                out=xk[0][:, :, 0:KC0],
                in_=xkTA[:, 0:KC0].rearrange("(k p) j -> p k j", p=128))
            dma_q_col(0, 0)
            nc.sync.dma_start(
                out=wv_sb.rearrange("p k j -> p (k j)"), in_=wv[:, :])
            # V-A in column chunks of 256 so early key tiles project early
            vchunks = [(c0, min(256, LCA - c0)) for c0 in range(0, LCA, 256)]
            nc.sync.dma_start(
                out=xv[0][:, :, 0:vchunks[0][1]],
                in_=xvTA[:, 0:vchunks[0][1]]
                .rearrange("(k p) j -> p k j", p=128))
            if LCA > KC0:
                nc.sync.dma_start(
                    out=xk[0][:, :, KC0:LCA],
                    in_=xkTA[:, KC0:LCA].rearrange("(k p) j -> p k j", p=128))
            for c0, cw in vchunks[1:]:
                nc.sync.dma_start(
                    out=xv[0][:, :, c0:c0 + cw],
                    in_=xvTA[:, c0:c0 + cw]
                    .rearrange("(k p) j -> p k j", p=128))
            dma_q_col(0, 1)
            nc.sync.dma_start(out=wo_sb, in_=wo[:, :])
            dma_q_col(0, 2)
            dma_q_col(0, 3)
            nc.sync.dma_start(
                out=xk[1][:, :, 0:LCB],
                in_=xkTB[:, 0:LCB].rearrange("(k p) j -> p k j", p=128))
            nc.sync.dma_start(
                out=xv[1][:, :, 0:LCB],
                in_=xvTB[:, 0:LCB].rearrange("(k p) j -> p k j", p=128))
            for ci in range(4):
                dma_q_col(1, ci)

            # ---- PE p-state warmup fills ----
            psW_cm = tc.tile_pool(name="psW", bufs=1, space="PSUM")
            psW = psW_cm.__enter__()
            wp = psW.tile([128, 256], f32)

            def wfill(n):
                for _ in range(n):
                    nc.tensor.matmul(wp, warm_sb[:, 0:128], warm_sb,
                                     start=True, stop=True)

            def copy_eng(eng, dst, src):
                if eng == "v":
                    nc.vector.tensor_copy(dst, src)
                elif eng == "a":
                    nc.scalar.copy(dst, src)
                else:
                    nc.gpsimd.tensor_copy(dst, src)

            wfill(18)

            def k_step(b, c0, cw, pool, eng="v"):
                # K^T projection for one column chunk through the shared
                # aux rotation
                acc = pool.tile([128, 512], f32, tag="x", name=f"xk{b}_{c0}")
                for k in range(KD):
                    nc.tensor.matmul(acc[:, 0:cw], wkq_sb[:, k, 0:128],
                                     xk[b][:, k, c0:c0 + cw],
                                     start=(k == 0), stop=(k == KD - 1))
                copy_eng(eng, kt_sb[b][:, c0:c0 + cw], acc[:, 0:cw])

            def qa_step(ci, b, pool, eng="v"):
                acc = pool.tile([128, 512], f32, tag="x", name=f"xq{b}_{ci}")
                for k in range(KD):
                    nc.tensor.matmul(acc, wkq_sb[:, k, 128:256],
                                     xq[b][:, k, ci * 512:(ci + 1) * 512],
                                     start=(k == 0), stop=(k == KD - 1))
                copy_eng(eng, qt_sb[b][:, ci * 512:(ci + 1) * 512], acc)

            def v_step(b, t, pool, eng=("v", "a")):
                # V'' projection for one key tile through the shared rotation
                acc = pool.tile([128, 512], f32, tag="x", name=f"v{b}_{t}")
                for k in range(KD):
                    nc.tensor.matmul(acc[:, 0:DL],
                                     xv[b][:, k, t * 128:(t + 1) * 128],
                                     wv_sb[:, k, :],
                                     start=(k == 0), stop=(k == KD - 1))
                for hh in range(2):
                    copy_eng(eng[hh % len(eng)], v_sb[b][:, t, hh, 0:64],
                             acc[:, hh * 64:(hh + 1) * 64])

            def a1b_step(pool):
                for i in range((LKB + 511) // 512):
                    c0 = i * 512
                    k_step(1, c0, min(512, LKB - c0), pool)

            evac_ct = [0]

            def out_proj(b, psD, op, qr, engines, dma="s"):
                # partial out[b] rows = ctx''[b]^T @ Wo_local; per query
                # tile. dma="d32": skip the SBUF evacuation and DMA the
                # fp32 PSUM halves straight to DRAM (pool relief; the DMA
                # device has the spare bandwidth).
                for qi in qr:
                    o_sb = op.tile([128, D], f16, tag=f"o{qi % 3}",
                                   name=f"osb{b}_{qi}")
                    for n in range(2):
                        o_ps = psD.tile([128, 512], f32, tag="x",
                                        name=f"o{b}_{qi}_{n}")
                        nc.tensor.matmul(o_ps,
                                         ctx_sb[b][:, qi * 128:(qi + 1) * 128],
                                         wo_sb[:, n * 512:(n + 1) * 512],
                                         start=True, stop=True)
                        eng = engines[evac_ct[0] % len(engines)]
                        evac_ct[0] += 1
                        copy_eng(eng, o_sb[:, n * 512:(n + 1) * 512], o_ps)
                    dq = nc.gpsimd if dma == "g" else nc.sync
                    dq.dma_start(
                        out=out[b, qi * 128:(qi + 1) * 128, :], in_=o_sb)

            def attend(b, pools, smap, ilv_first=False, fin=None,
                       chunks=range(4), tail_groups=()):
                # scores^T -> exp -> [extras] -> ctx'' per (512-query-chunk,
                # head, key-tile). ilv_first interleaves both heads of chunk
                # 0 to cover the V-stream wait; fin(j) fires after each
                # 128-query fine-norm slice of the very last group so the
                # final out tiles pipeline instead of waiting a full norm.
                ptp, mp, psS, psC = pools
                nit = [0]
                pts = {}

                def norm2(b, ctx_ps, hh, q0, fine=None):
                    # rows 64-127 all hold the softmax denominator
                    if fine is None:
                        rcb = mp.tile([64, 512], f32, tag="rcb")
                        nc.vector.reciprocal(rcb, ctx_ps[64:128, :])
                        nc.vector.tensor_mul(
                            ctx_sb[b][hh * 64:hh * 64 + 64, q0:q0 + 512],
                            ctx_ps[0:64, :], rcb)
                        return
                    for j in range(4):
                        c0 = j * 128
                        rcb = mp.tile([64, 128], f32, tag="rcf")
                        nc.vector.reciprocal(rcb, ctx_ps[64:128, c0:c0 + 128])
                        nc.vector.tensor_mul(
                            ctx_sb[b][hh * 64:hh * 64 + 64,
                                      q0 + c0:q0 + c0 + 128],
                            ctx_ps[0:64, c0:c0 + 128], rcb)
                        fine(j)

                def sc2(b, ctx_ps, hh, q0, t, phase):
                    if phase in ("s", "sc"):
                        s_ps = psS.tile([128, 512], f32, tag="s",
                                        name=f"s{b}_{hh}_{q0}_{t}")
                        nc.tensor.matmul(
                            s_ps,
                            kt_sb[b][hh * 64:hh * 64 + 64,
                                     t * 128:(t + 1) * 128],
                            qt_sb[b][hh * 64:hh * 64 + 64, q0:q0 + 512],
                            start=True, stop=True)
                        pt = ptp.tile([128, 512], f16, tag="pt",
                                      name=f"pt{b}_{hh}_{q0}_{t}")
                        pts[(b, hh, t)] = pt
                        nc.scalar.activation(
                            pt, s_ps, EXP,
                            bias=mask_sb[b][:, t:t + 1], scale=0.125)
                    if phase in ("c", "sc"):
                        for fn in smap.get(nit[0], ()):
                            fn()
                        nc.tensor.matmul(
                            ctx_ps, v_sb[b][:, t, hh, :], pts.pop((b, hh, t)),
                            start=(t == 0), stop=(t == KT[b] - 1),
                            skip_group_check=True)
                        nit[0] += 1

                def group(b, cq, hh):
                    q0 = cq * 512
                    ctx_ps = psC.tile([128, 512], f32, tag="ctx",
                                      name=f"ctx{b}_{hh}_{cq}")
                    for t in range(KT[b]):
                        sc2(b, ctx_ps, hh, q0, t, "sc")
                    norm2(b, ctx_ps, hh, q0,
                          fine=fin if (fin and cq == 3 and hh == 1)
                          else None)

                for cq in chunks:
                    q0 = cq * 512
                    if ilv_first and cq == 0:
                        cps = {hh: psC.tile([128, 512], f32, tag="ctx",
                                            name=f"ctx{b}_{hh}_0")
                               for hh in range(2)}
                        for t in range(KT[b]):
                            sc2(b, cps[0], 0, q0, t, "s")
                            sc2(b, cps[1], 1, q0, t, "s")
                            sc2(b, cps[0], 0, q0, t, "c")
                            sc2(b, cps[1], 1, q0, t, "c")
                        norm2(b, cps[0], 0, q0)
                        norm2(b, cps[1], 1, q0)
                        continue
                    for hh in range(2):
                        group(b, cq, hh)
                for b2, cq2, hh2 in tail_groups:
                    group(b2, cq2, hh2)

            psW_cm.__exit__(None, None, None)

            # ---- attention + everything else, one PSUM configuration ----
            with tc.tile_pool(name="pt", bufs=8) as ptp, \
                 tc.tile_pool(name="misc", bufs=4) as mp, \
                 tc.tile_pool(name="ob", bufs=6) as op:
                with tc.tile_pool(name="psS", bufs=4, space="PSUM") as psS, \
                     tc.tile_pool(name="psC", bufs=2, space="PSUM") as psC, \
                     tc.tile_pool(name="aux", bufs=2, space="PSUM") as aux:
                    # K-proj A chunk 0 + Q-proj A chunk 0 gate the
                    # first scores; V tiles + K tail ride the early cadence
                    KC0 = min(512, LKA)
                    k_step(0, 0, KC0, aux, eng="a")
                    qa_step(0, 0, aux)

                    nit_A = 8 * KT[0]
                    smap = {}

                    def put(it, fn):
                        smap.setdefault(min(it, nit_A - 1), []).append(fn)

                    put(0, lambda: v_step(0, 0, aux))
                    put(1, lambda: v_step(0, 1, aux))
                    if LKA > KC0:
                        put(2, lambda: k_step(0, KC0, LKA - KC0, aux))
                    for gi, t in enumerate(range(2, KT[0])):
                        put(3 + gi, lambda t=t: v_step(0, t, aux))
                    # Q-proj A chunks 1-3 track the xq column-DMA arrivals
                    put(8, lambda: qa_step(1, 0, aux))
                    put(12, lambda: qa_step(2, 0, aux))
                    put(15, lambda: qa_step(3, 0, aux))
                    # batch-B projections ride the middle of attend(0)
                    put(17, lambda: a1b_step(aux))
                    for j in range(KT[1]):
                        put(19 + j, lambda t=j: v_step(1, t, aux, ("v", "v")))
                    put(23, lambda: qa_step(0, 1, aux))
                    put(26, lambda: qa_step(1, 1, aux))
                    put(29, lambda: qa_step(2, 1, aux))
                    put(33, lambda: qa_step(3, 1, aux))
                    # batch-A out-proj: chunk c's query tiles as soon as its
                    # softmax normalizes (after iteration (2c+2)*KT)
                    oiters = {0: (11, 13, 14, 16), 1: (22, 24, 25, 27),
                              2: (31, 32, 34, 35)}
                    for c, its in oiters.items():
                        for j, it in enumerate(its):
                            put(max(it, (2 * c + 2) * KT[0] + 1),
                                lambda qi=4 * c + j: out_proj(
                                    0, aux, op, [qi], ("v", "a", "v")))

                    attend(0, (ptp, mp, psS, psC), smap, ilv_first=True,
                           tail_groups=((1, 0, 0), (1, 0, 1)))

                    # attend(1): batch-A chunk 3 + batch-B chunks as ready
                    nit_B = 6 * KT[1]
                    smapB = {}

                    def putB(it, fn):
                        smapB.setdefault(min(it, nit_B - 1), []).append(fn)

                    # batch-A chunk 3 + batch-B chunk 0 tiles lead off
                    for j, (b_, qi) in enumerate(
                            ((0, 12), (0, 13), (1, 0), (0, 14), (0, 15),
                             (1, 1), (1, 2), (1, 3))):
                        putB(j, lambda b_=b_, qi=qi: out_proj(
                            b_, aux, op, [qi], ("v", "a")))
                    for c in (1, 2):
                        rdy = 2 * c * KT[1] + 2
                        for j, qi in enumerate(range(4 * c, 4 * c + 4)):
                            putB(rdy + j, lambda qi=qi: out_proj(
                                1, aux, op, [qi], ("a", "v")))

                    attend(1, (ptp, mp, psS, psC), smapB,
                           chunks=range(1, 4))
                    out_proj(1, aux, op, [12], ("a", "v"), dma="g")
                with tc.tile_pool(name="psD2", bufs=6, space="PSUM") as psD2:
                    out_proj(1, psD2, op, [13], ("v", "a"))
                    out_proj(1, psD2, op, [14], ("a", "v"), dma="g")
                    out_proj(1, psD2, op, [15], ("v", "a"))
            strm_cm.__exit__(None, None, None)
    nc.compile()
    return nc


def kernel(**inputs):
    global last_results, last_exec_wall_s
    from concourse.bass_utils import run_bass_kernel_spmd

    # BASS_TRACE needs the axon NTFF hook; disable tracing when the hook
    # module is unavailable so a stray env var cannot crash the run.
    if os.environ.get("BASS_TRACE"):
        try:
            from antenv import axon_hooks  # noqa: F401
        except Exception:
            os.environ["BASS_NEVER_TRACE"] = "1"

    q = np.asarray(inputs["queries"], dtype=np.float32)
    kx = np.asarray(inputs["keys"], dtype=np.float32)
    vx = np.asarray(inputs["values"], dtype=np.float32)
    vl = np.asarray(inputs["valid_lens"], dtype=np.int64).reshape(B)
    Wq = np.asarray(inputs["Wq"], dtype=np.float32)
    Wk = np.asarray(inputs["Wk"], dtype=np.float32)
    Wv = np.asarray(inputs["Wv"], dtype=np.float32)
    Wo = np.asarray(inputs["Wo"], dtype=np.float32)
    assert q.shape == (B, SQ, D) and kx.shape == (B, SK, D) and vx.shape == (B, SK, D)

    lens = np.clip(vl, 1, SK)
    KTs = [(int(l) + 127) // 128 for l in lens]
    # batch A = more key tiles, processed first
    bA = 0 if KTs[0] >= KTs[1] else 1
    bB = 1 - bA
    KTA, KTB = KTs[bA], KTs[bB]
    LKA, LKB = KTA * 128, KTB * 128

    LCA = min(LKA, -(-int(lens[bA]) // 8) * 8)
    LCB = min(LKB, -(-int(lens[bB]) // 8) * 8)
    key = (KTA, KTB, LCA, LCB)
    if key not in _NC_CACHE:
        _NC_CACHE[key] = _build(KTA, KTB, LCA, LCB)
    nc = _NC_CACHE[key]

    def m128(b, KT):
        m = np.where(np.arange(KT * 128) < lens[b], 0.0, NEG).astype(np.float32)
        return np.ascontiguousarray(m.reshape(KT, 128).T)

    def pack_pm(w):
        # [1024, C] -> partition-major [128, KD*C] (p, k, j)
        c = w.shape[1]
        return np.ascontiguousarray(
            w.reshape(KD, 128, c).transpose(1, 0, 2).reshape(128, KD * c)
            .astype(np.float16))

    xqT_full = np.ascontiguousarray(
        np.stack([q[bA].T, q[bB].T]).astype(np.float16))
    in_maps = []
    for c in range(N_CORES):
        cols = slice(DL * c, DL * (c + 1))
        in_maps.append({
            "xqT": xqT_full,
            "xkTA": np.ascontiguousarray(kx[bA, :LKA].T.astype(np.float16)),
            "xvTA": np.ascontiguousarray(vx[bA, :LKA].T.astype(np.float16)),
            "xkTB": np.ascontiguousarray(kx[bB, :LKB].T.astype(np.float16)),
            "xvTB": np.ascontiguousarray(vx[bB, :LKB].T.astype(np.float16)),
            "wkq": pack_pm(np.concatenate([Wk[:, cols], Wq[:, cols]], axis=1)),
            "wv": pack_pm(Wv[:, cols]),
            "wo": np.ascontiguousarray(Wo[cols, :].astype(np.float16)),
            "maskA": m128(bA, KTA),
            "maskB": m128(bB, KTB),
        })

    t0 = time.perf_counter()
    res = run_bass_kernel_spmd(nc, in_maps, core_ids=list(range(N_CORES)))
    last_exec_wall_s = time.perf_counter() - t0
    last_results = res

    outs = [res.results[c]["out"].astype(np.float32) for c in range(N_CORES)]
    acc = outs[0]
    for c in range(1, N_CORES):
        acc = acc + outs[c]
    full = np.empty((B, SQ, D), dtype=np.float32)
    full[bA] = acc[0]
    full[bB] = acc[1]
    return full


# revision 74
# speedup vs baseline: 1.0051x; 1.0051x over previous
"""Multi-head attention (B=2, S=2048, D=1024, H=16) on 8 Trainium2 cores.

Sharding: pure tensor-parallel over heads (Megatron): core c owns heads
{2c, 2c+1} (d_local = 128 columns of Wq/Wk/Wv, 128 rows of Wo) and
processes BOTH batches. Each core emits a [2, 2048, 1024] partial output
(row-parallel Wo); the host sums the 8 partials per batch.

v2 schedule (86.7us vs the 89.9us baseline, TimelineSim cost model):
- A dummy exp at t~0 pulls the 1.3us activation-table load off the
  first real exp's critical path; the tiny mask DMAs go out on the
  GPSIMD SWDGE queue ahead of the Pool memsets so they beat the big
  stream transfers into the serial DMA queue.
- DMA order puts the attention-gating tensors first (Wk+Wq packed
  partition-major in one DMA, K-A, Q-A chunk 0); V streams in column
  chunks and all projections run as single-PSUM-bank steps through a
  shared 2-bank aux rotation, scheduled into the attention cadence by
  iteration index (smap).
- attend(0) processes batch A with both heads of chunk 0 interleaved
  (early scores cover the V-stream wait) and batch B's chunk 0
  appended as tail groups; attend(1) covers B chunks 1-3. Out-proj
  tiles fire as each chunk's softmax normalizes, spread so the final
  phase stays below the ACT+DVE evacuation capacity; only chunk B-c3's
  4 tiles drain through a deep PSUM rotation after the loop.
- Inner attention loop emits [scores(t) | exp | extras | ctx(t)] so
  foreign PE work hides the exp latency.

Precision: fp16 streams/weights, fp32 PSUM accumulation (rel err ~8e-4).
PSUM: scores rotation 4 + ctx accumulators 2 + shared aux 2 = 8 banks.
"""
import sys
if "/opt/trn_rl_repo" not in sys.path:
    sys.path.insert(0, "/opt/trn_rl_repo")
import os
import time
import numpy as np

B, SQ, SK, D, H, HD = 2, 2048, 2048, 1024, 16, 64
NEG = -1.0e6
N_CORES = 8
DL = 128          # d_local: 2 heads * 64
KD = D // 128     # contraction tiles over D

_NC_CACHE = {}
last_results = None
last_exec_wall_s = None


def _build(KTA, KTB, LCA=None, LCB=None):
    import concourse.bass as bass  # noqa: F401
    import concourse.tile as tile
    from concourse import bacc, mybir

    f32 = mybir.dt.float32
    f16 = mybir.dt.float16
    EXP = mybir.ActivationFunctionType.Exp

    LKA, LKB = KTA * 128, KTB * 128
    LCA = LKA if LCA is None else LCA
    LCB = LKB if LCB is None else LCB

    nc = bacc.Bacc("TRN2", target_bir_lowering=False, debug=False,
                   num_devices=N_CORES)
    xqT = nc.dram_tensor("xqT", [2, D, SQ], f16, kind="ExternalInput")
    xkTA = nc.dram_tensor("xkTA", [D, LKA], f16, kind="ExternalInput")
    xvTA = nc.dram_tensor("xvTA", [D, LKA], f16, kind="ExternalInput")
    xkTB = nc.dram_tensor("xkTB", [D, LKB], f16, kind="ExternalInput")
    xvTB = nc.dram_tensor("xvTB", [D, LKB], f16, kind="ExternalInput")
    # host-packed partition-major weights: one big contiguous DMA each
    wkq = nc.dram_tensor("wkq", [128, KD * 256], f16, kind="ExternalInput")
    wv = nc.dram_tensor("wv", [128, KD * 128], f16, kind="ExternalInput")
    wo = nc.dram_tensor("wo", [DL, D], f16, kind="ExternalInput")
    maskA = nc.dram_tensor("maskA", [128, KTA], f32, kind="ExternalInput")
    maskB = nc.dram_tensor("maskB", [128, KTB], f32, kind="ExternalInput")
    out = nc.dram_tensor("out", [2, SQ, D], f16, kind="ExternalOutput")

    with tile.TileContext(nc) as tc:
        with tc.tile_pool(name="singles", bufs=1) as sg:
            wkq_sb = sg.tile([128, KD, 256], f16)
            wv_sb = sg.tile([128, KD, 128], f16)
            wo_sb = sg.tile([128, D], f16)
            maskA_sb = sg.tile([128, KTA], f32)
            maskB_sb = sg.tile([128, KTB], f32)
            kt_sb = {0: sg.tile([128, LKA], f16, name="ktA"),
                     1: sg.tile([128, LKB], f16, name="ktB")}
            qt_sb = {0: sg.tile([128, SQ], f16, name="qtA"),
                     1: sg.tile([128, SQ], f16, name="qtB")}
            v_sb = {0: sg.tile([128, KTA, 2, 128], f16, name="vA"),
                    1: sg.tile([128, KTB, 2, 128], f16, name="vB")}
            ctx_sb = {0: sg.tile([128, SQ], f16, name="ctxA"),
                      1: sg.tile([128, SQ], f16, name="ctxB")}
            warm_sb = sg.tile([128, 256], f16)

            KT = {0: KTA, 1: KTB}
            mask_sb = {0: maskA_sb, 1: maskB_sb}

            strm_cm = tc.tile_pool(name="streams", bufs=1)
            strm = strm_cm.__enter__()
            xk = {b: strm.tile([128, KD, 128 * KT[b]], f16, name=f"xk{b}")
                  for b in (0, 1)}
            xq = {b: strm.tile([128, KD, SQ], f16, name=f"xq{b}")
                  for b in (0, 1)}
            xv = {b: strm.tile([128, KD, 128 * KT[b]], f16, name=f"xv{b}")
                  for b in (0, 1)}
            LC = {0: LCA, 1: LCB}

            # ---- GPSIMD preamble: warm tile first (gates PE fills and the
            # ACT-table prewarm), then the tiny mask DMAs (so their SWDGE
            # descriptors beat the big stream transfers into the DMA queue),
            # then V'' ones-columns and stream tail zero-fill ----
            nc.vector.memset(warm_sb, 0.0)
            nc.gpsimd.dma_start(out=maskA_sb, in_=maskA[:, :])
            nc.gpsimd.dma_start(out=maskB_sb, in_=maskB[:, :])
            nc.gpsimd.memset(v_sb[0], 1.0)
            nc.gpsimd.memset(v_sb[1], 1.0)
            for b in (0, 1):
                if LC[b] < 128 * KT[b]:
                    nc.gpsimd.memset(xk[b][:, :, LC[b]:], 0.0)
                    nc.gpsimd.memset(xv[b][:, :, LC[b]:], 0.0)
            # dummy exp at t~0: pulls the 1.3us activation-table load off
            # the first real exp's critical path
            nc.scalar.activation(warm_sb[:, 255:256], warm_sb[:, 0:1], EXP,
                                 scale=1.0)

            # ---- SP/HWDGE input DMAs, arrival order = need order ----
            def dma_q_col(b, ci):
                nc.sync.dma_start(
                    out=xq[b][:, :, ci * 512:(ci + 1) * 512],
                    in_=xqT[b, :, ci * 512:(ci + 1) * 512]
                    .rearrange("(k p) j -> p k j", p=128))

            nc.sync.dma_start(
                out=wkq_sb.rearrange("p k j -> p (k j)"), in_=wkq[:, :])
            KC0 = min(512, LCA)
            nc.sync.dma_start(
                out=xk[0][:, :, 0:KC0],
                in_=xkTA[:, 0:KC0].rearrange("(k p) j -> p k j", p=128))
            # chunk 0 of Q in two k-halves: Q-proj k0-3 overlaps the
            # second half's transfer, pulling the first scores earlier
            nc.sync.dma_start(
                out=xq[0][:, 0:4, 0:512],
                in_=xqT[0, 0:512, 0:512]
                .rearrange("(k p) j -> p k j", p=128))
            nc.sync.dma_start(
                out=xq[0][:, 4:8, 0:512],
                in_=xqT[0, 512:1024, 0:512]
                .rearrange("(k p) j -> p k j", p=128))
            nc.sync.dma_start(
                out=wv_sb.rearrange("p k j -> p (k j)"), in_=wv[:, :])
            # V-A in column chunks of 256 so early key tiles project early
            vchunks = [(c0, min(256, LCA - c0)) for c0 in range(0, LCA, 256)]
            nc.sync.dma_start(
                out=xv[0][:, :, 0:vchunks[0][1]],
                in_=xvTA[:, 0:vchunks[0][1]]
                .rearrange("(k p) j -> p k j", p=128))
            if LCA > KC0:
                nc.sync.dma_start(
                    out=xk[0][:, :, KC0:LCA],
                    in_=xkTA[:, KC0:LCA].rearrange("(k p) j -> p k j", p=128))
            for c0, cw in vchunks[1:]:
                nc.sync.dma_start(
                    out=xv[0][:, :, c0:c0 + cw],
                    in_=xvTA[:, c0:c0 + cw]
                    .rearrange("(k p) j -> p k j", p=128))
            nc.sync.dma_start(
                out=xq[0][:, 0:4, 512:1024],
                in_=xqT[0, 0:512, 512:1024]
                .rearrange("(k p) j -> p k j", p=128))
            nc.sync.dma_start(
                out=xq[0][:, 4:8, 512:1024],
                in_=xqT[0, 512:1024, 512:1024]
                .rearrange("(k p) j -> p k j", p=128))
            nc.sync.dma_start(out=wo_sb, in_=wo[:, :])
            dma_q_col(0, 2)
            dma_q_col(0, 3)
            nc.sync.dma_start(
                out=xk[1][:, :, 0:LCB],
                in_=xkTB[:, 0:LCB].rearrange("(k p) j -> p k j", p=128))
            nc.sync.dma_start(
                out=xv[1][:, :, 0:LCB],
                in_=xvTB[:, 0:LCB].rearrange("(k p) j -> p k j", p=128))
            for ci in range(4):
                dma_q_col(1, ci)

            # ---- PE p-state warmup fills ----
            psW_cm = tc.tile_pool(name="psW", bufs=1, space="PSUM")
            psW = psW_cm.__enter__()
            wp = psW.tile([128, 256], f32)

            def wfill(n):
                for _ in range(n):
                    nc.tensor.matmul(wp, warm_sb[:, 0:128], warm_sb,
                                     start=True, stop=True)

            def copy_eng(eng, dst, src):
                if eng == "v":
                    nc.vector.tensor_copy(dst, src)
                elif eng == "a":
                    nc.scalar.copy(dst, src)
                else:
                    nc.gpsimd.tensor_copy(dst, src)

            wfill(18)

            def k_step(b, c0, cw, pool, eng="v"):
                # K^T projection for one column chunk through the shared
                # aux rotation
                acc = pool.tile([128, 512], f32, tag="x", name=f"xk{b}_{c0}")
                for k in range(KD):
                    nc.tensor.matmul(acc[:, 0:cw], wkq_sb[:, k, 0:128],
                                     xk[b][:, k, c0:c0 + cw],
                                     start=(k == 0), stop=(k == KD - 1))
                copy_eng(eng, kt_sb[b][:, c0:c0 + cw], acc[:, 0:cw])

            def qa_step(ci, b, pool, eng="v"):
                acc = pool.tile([128, 512], f32, tag="x", name=f"xq{b}_{ci}")
                for k in range(KD):
                    nc.tensor.matmul(acc, wkq_sb[:, k, 128:256],
                                     xq[b][:, k, ci * 512:(ci + 1) * 512],
                                     start=(k == 0), stop=(k == KD - 1))
                copy_eng(eng, qt_sb[b][:, ci * 512:(ci + 1) * 512], acc)

            def v_step(b, t, pool, eng=("v", "a")):
                # V'' projection for one key tile through the shared rotation
                acc = pool.tile([128, 512], f32, tag="x", name=f"v{b}_{t}")
                for k in range(KD):
                    nc.tensor.matmul(acc[:, 0:DL],
                                     xv[b][:, k, t * 128:(t + 1) * 128],
                                     wv_sb[:, k, :],
                                     start=(k == 0), stop=(k == KD - 1))
                for hh in range(2):
                    copy_eng(eng[hh % len(eng)], v_sb[b][:, t, hh, 0:64],
                             acc[:, hh * 64:(hh + 1) * 64])

            def a1b_step(pool):
                for i in range((LKB + 511) // 512):
                    c0 = i * 512
                    k_step(1, c0, min(512, LKB - c0), pool)

            evac_ct = [0]

            def out_proj(b, psD, op, qr, engines, dma="s"):
                # partial out[b] rows = ctx''[b]^T @ Wo_local; per query
                # tile. dma="d32": skip the SBUF evacuation and DMA the
                # fp32 PSUM halves straight to DRAM (pool relief; the DMA
                # device has the spare bandwidth).
                for qi in qr:
                    o_sb = op.tile([128, D], f16, tag=f"o{qi % 3}",
                                   name=f"osb{b}_{qi}")
                    for n in range(2):
                        o_ps = psD.tile([128, 512], f32, tag="x",
                                        name=f"o{b}_{qi}_{n}")
                        nc.tensor.matmul(o_ps,
                                         ctx_sb[b][:, qi * 128:(qi + 1) * 128],
                                         wo_sb[:, n * 512:(n + 1) * 512],
                                         start=True, stop=True)
                        eng = engines[evac_ct[0] % len(engines)]
                        evac_ct[0] += 1
                        copy_eng(eng, o_sb[:, n * 512:(n + 1) * 512], o_ps)
                    dq = nc.gpsimd if dma == "g" else nc.sync
                    dq.dma_start(
                        out=out[b, qi * 128:(qi + 1) * 128, :], in_=o_sb)

            def attend(b, pools, smap, ilv_first=False, fin=None,
                       chunks=range(4), tail_groups=()):
                # scores^T -> exp -> [extras] -> ctx'' per (512-query-chunk,
                # head, key-tile). ilv_first interleaves both heads of chunk
                # 0 to cover the V-stream wait; fin(j) fires after each
                # 128-query fine-norm slice of the very last group so the
                # final out tiles pipeline instead of waiting a full norm.
                ptp, mp, psS, psC = pools
                nit = [0]
                pts = {}

                def norm2(b, ctx_ps, hh, q0, fine=None):
                    # rows 64-127 all hold the softmax denominator
                    if fine is None:
                        rcb = mp.tile([64, 512], f32, tag="rcb")
                        nc.vector.reciprocal(rcb, ctx_ps[64:128, :])
                        nc.vector.tensor_mul(
                            ctx_sb[b][hh * 64:hh * 64 + 64, q0:q0 + 512],
                            ctx_ps[0:64, :], rcb)
                        return
                    for j in range(4):
                        c0 = j * 128
                        rcb = mp.tile([64, 128], f32, tag="rcf")
                        nc.vector.reciprocal(rcb, ctx_ps[64:128, c0:c0 + 128])
                        nc.vector.tensor_mul(
                            ctx_sb[b][hh * 64:hh * 64 + 64,
                                      q0 + c0:q0 + c0 + 128],
                            ctx_ps[0:64, c0:c0 + 128], rcb)
                        fine(j)

                def sc2(b, ctx_ps, hh, q0, t, phase):
                    if phase in ("s", "sc"):
                        s_ps = psS.tile([128, 512], f32, tag="s",
                                        name=f"s{b}_{hh}_{q0}_{t}")
                        nc.tensor.matmul(
                            s_ps,
                            kt_sb[b][hh * 64:hh * 64 + 64,
                                     t * 128:(t + 1) * 128],
                            qt_sb[b][hh * 64:hh * 64 + 64, q0:q0 + 512],
                            start=True, stop=True)
                        pt = ptp.tile([128, 512], f16, tag="pt",
                                      name=f"pt{b}_{hh}_{q0}_{t}")
                        pts[(b, hh, t)] = pt
                        nc.scalar.activation(
                            pt, s_ps, EXP,
                            bias=mask_sb[b][:, t:t + 1], scale=0.125)
                    if phase in ("c", "sc"):
                        for fn in smap.get(nit[0], ()):
                            fn()
                        nc.tensor.matmul(
                            ctx_ps, v_sb[b][:, t, hh, :], pts.pop((b, hh, t)),
                            start=(t == 0), stop=(t == KT[b] - 1),
                            skip_group_check=True)
                        nit[0] += 1

                def group(b, cq, hh):
                    q0 = cq * 512
                    ctx_ps = psC.tile([128, 512], f32, tag="ctx",
                                      name=f"ctx{b}_{hh}_{cq}")
                    for t in range(KT[b]):
                        sc2(b, ctx_ps, hh, q0, t, "sc")
                    norm2(b, ctx_ps, hh, q0,
                          fine=fin if (fin and cq == 3 and hh == 1)
                          else None)

                for cq in chunks:
                    q0 = cq * 512
                    if ilv_first and cq == 0:
                        cps = {hh: psC.tile([128, 512], f32, tag="ctx",
                                            name=f"ctx{b}_{hh}_0")
                               for hh in range(2)}
                        for t in range(KT[b]):
                            sc2(b, cps[0], 0, q0, t, "s")
                            sc2(b, cps[1], 1, q0, t, "s")
                            sc2(b, cps[0], 0, q0, t, "c")
                            sc2(b, cps[1], 1, q0, t, "c")
                        norm2(b, cps[0], 0, q0)
                        norm2(b, cps[1], 1, q0)
                        continue
                    for hh in range(2):
                        group(b, cq, hh)
                for b2, cq2, hh2 in tail_groups:
                    group(b2, cq2, hh2)

            psW_cm.__exit__(None, None, None)

            # ---- attention + everything else, one PSUM configuration ----
            with tc.tile_pool(name="pt", bufs=8) as ptp, \
                 tc.tile_pool(name="misc", bufs=4) as mp, \
                 tc.tile_pool(name="ob", bufs=6) as op:
                with tc.tile_pool(name="psS", bufs=4, space="PSUM") as psS, \
                     tc.tile_pool(name="psC", bufs=2, space="PSUM") as psC, \
                     tc.tile_pool(name="aux", bufs=2, space="PSUM") as aux:
                    # K-proj A chunk 0 + Q-proj A chunk 0 gate the
                    # first scores; V tiles + K tail ride the early cadence
                    KC0 = min(512, LKA)
                    k_step(0, 0, KC0, aux, eng="a")
                    qa_step(0, 0, aux)

                    nit_A = 8 * KT[0]
                    smap = {}

                    def put(it, fn):
                        smap.setdefault(min(it, nit_A - 1), []).append(fn)

                    put(0, lambda: v_step(0, 0, aux))
                    put(1, lambda: v_step(0, 1, aux))
                    if LKA > KC0:
                        put(2, lambda: k_step(0, KC0, LKA - KC0, aux))
                    for gi, t in enumerate(range(2, KT[0])):
                        put(3 + gi, lambda t=t: v_step(0, t, aux))
                    # Q-proj A chunks 1-3 track the xq column-DMA arrivals
                    put(8, lambda: qa_step(1, 0, aux))
                    put(12, lambda: qa_step(2, 0, aux))
                    put(15, lambda: qa_step(3, 0, aux))
                    # batch-B projections ride the middle of attend(0)
                    put(17, lambda: a1b_step(aux))
                    for j in range(KT[1]):
                        put(19 + j, lambda t=j: v_step(1, t, aux, ("v", "v")))
                    put(23, lambda: qa_step(0, 1, aux))
                    put(26, lambda: qa_step(1, 1, aux))
                    put(29, lambda: qa_step(2, 1, aux))
                    put(33, lambda: qa_step(3, 1, aux))
                    # batch-A out-proj: chunk c's query tiles as soon as its
                    # softmax normalizes (after iteration (2c+2)*KT)
                    oiters = {0: (11, 13, 14, 16), 1: (22, 24, 25, 27),
                              2: (31, 32, 34, 35)}
                    for c, its in oiters.items():
                        for j, it in enumerate(its):
                            put(max(it, (2 * c + 2) * KT[0] + 1),
                                lambda qi=4 * c + j: out_proj(
                                    0, aux, op, [qi], ("v", "a", "v")))

                    attend(0, (ptp, mp, psS, psC), smap, ilv_first=True,
                           tail_groups=((1, 0, 0), (1, 0, 1)))

                    # attend(1): batch-A chunk 3 + batch-B chunks as ready
                    nit_B = 6 * KT[1]
                    smapB = {}

                    def putB(it, fn):
                        smapB.setdefault(min(it, nit_B - 1), []).append(fn)

                    # batch-A chunk 3 + batch-B chunk 0 tiles lead off
                    for j, (b_, qi) in enumerate(
                            ((0, 12), (0, 13), (1, 0), (0, 14), (0, 15),
                             (1, 1), (1, 2), (1, 3))):
                        putB(j, lambda b_=b_, qi=qi: out_proj(
                            b_, aux, op, [qi], ("v", "a")))
                    for c in (1, 2):
                        rdy = 2 * c * KT[1] + 2
                        for j, qi in enumerate(range(4 * c, 4 * c + 4)):
                            putB(rdy + j, lambda qi=qi: out_proj(
                                1, aux, op, [qi], ("a", "v")))

                    attend(1, (ptp, mp, psS, psC), smapB,
                           chunks=range(1, 4))
                    out_proj(1, aux, op, [12], ("a", "v"), dma="g")
                with tc.tile_pool(name="psD2", bufs=6, space="PSUM") as psD2:
                    out_proj(1, psD2, op, [13], ("v", "a"))
                    out_proj(1, psD2, op, [14], ("a", "v"), dma="g")
                    out_proj(1, psD2, op, [15], ("v", "a"))
            strm_cm.__exit__(None, None, None)
    nc.compile()
    return nc


def kernel(**inputs):
    global last_results, last_exec_wall_s
    from concourse.bass_utils import run_bass_kernel_spmd

    # BASS_TRACE needs the axon NTFF hook; disable tracing when the hook
    # module is unavailable so a stray env var cannot crash the run.
    if os.environ.get("BASS_TRACE"):
        try:
            from antenv import axon_hooks  # noqa: F401
        except Exception:
            os.environ["BASS_NEVER_TRACE"] = "1"

    q = np.asarray(inputs["queries"], dtype=np.float32)
    kx = np.asarray(inputs["keys"], dtype=np.float32)
    vx = np.asarray(inputs["values"], dtype=np.float32)
    vl = np.asarray(inputs["valid_lens"], dtype=np.int64).reshape(B)
    Wq = np.asarray(inputs["Wq"], dtype=np.float32)
    Wk = np.asarray(inputs["Wk"], dtype=np.float32)
    Wv = np.asarray(inputs["Wv"], dtype=np.float32)
    Wo = np.asarray(inputs["Wo"], dtype=np.float32)
    assert q.shape == (B, SQ, D) and kx.shape == (B, SK, D) and vx.shape == (B, SK, D)

    lens = np.clip(vl, 1, SK)
    KTs = [(int(l) + 127) // 128 for l in lens]
    # batch A = more key tiles, processed first
    bA = 0 if KTs[0] >= KTs[1] else 1
    bB = 1 - bA
    KTA, KTB = KTs[bA], KTs[bB]
    LKA, LKB = KTA * 128, KTB * 128

    LCA = min(LKA, -(-int(lens[bA]) // 8) * 8)
    LCB = min(LKB, -(-int(lens[bB]) // 8) * 8)
    key = (KTA, KTB, LCA, LCB)
    if key not in _NC_CACHE:
        _NC_CACHE[key] = _build(KTA, KTB, LCA, LCB)
    nc = _NC_CACHE[key]

    def m128(b, KT):
        m = np.where(np.arange(KT * 128) < lens[b], 0.0, NEG).astype(np.float32)
        return np.ascontiguousarray(m.reshape(KT, 128).T)

    def pack_pm(w):
        # [1024, C] -> partition-major [128, KD*C] (p, k, j)
        c = w.shape[1]
        return np.ascontiguousarray(
            w.reshape(KD, 128, c).transpose(1, 0, 2).reshape(128, KD * c)
            .astype(np.float16))

    xqT_full = np.ascontiguousarray(
        np.stack([q[bA].T, q[bB].T]).astype(np.float16))
    in_maps = []
    for c in range(N_CORES):
        cols = slice(DL * c, DL * (c + 1))
        in_maps.append({
            "xqT": xqT_full,
            "xkTA": np.ascontiguousarray(kx[bA, :LKA].T.astype(np.float16)),
            "xvTA": np.ascontiguousarray(vx[bA, :LKA].T.astype(np.float16)),
            "xkTB": np.ascontiguousarray(kx[bB, :LKB].T.astype(np.float16)),
            "xvTB": np.ascontiguousarray(vx[bB, :LKB].T.astype(np.float16)),
            "wkq": pack_pm(np.concatenate([Wk[:, cols], Wq[:, cols]], axis=1)),
            "wv": pack_pm(Wv[:, cols]),
            "wo": np.ascontiguousarray(Wo[cols, :].astype(np.float16)),
            "maskA": m128(bA, KTA),
            "maskB": m128(bB, KTB),
        })

    t0 = time.perf_counter()
    res = run_bass_kernel_spmd(nc, in_maps, core_ids=list(range(N_CORES)))
    last_exec_wall_s = time.perf_counter() - t0
    last_results = res

    outs = [res.results[c]["out"].astype(np.float32) for c in range(N_CORES)]
    acc = outs[0]
    for c in range(1, N_CORES):
        acc = acc + outs[c]
    full = np.empty((B, SQ, D), dtype=np.float32)
    full[bA] = acc[0]
    full[bB] = acc[1]
    return full
